# revision 34
# baseline (speedup 1.0000x reference)
"""
Distributed GQA attention block for Trainium2 (8 NeuronCores).

Problem: out = AttentionBlock(x; wq, wk, wv, wo)
  B=2, S=2048, DIM=4096, n_heads=32, n_kv_heads=8, head_dim=128,
  rope theta=5e5, causal, softmax, f32 I/O.

Sharding strategy (tensor-parallel over heads, AllGather instead of
AllReduce):
  - Each core c owns 4 query heads (4c..4c+3) and 1 kv head (c).
  - Per core: q/k/v projections for its heads (column shards of wq/wk/wv),
    RoPE, causal attention for its 4 heads over the full sequence.
  - The per-core attention output ([tokens, 512], stored transposed
    [512, tokens] in bf16) is AllGather'd (4 MB/rank) -> every core holds
    the full [4096 feat, 4096 tok] attention output.
  - The output projection is COLUMN-sharded: core c computes
    out[:, 512c:512c+512] = attn_full @ wo[:, 512c:512c+512].
    Host-side unshard is a pure concatenation along features (no host
    compute), so no AllReduce is needed anywhere.

Compute dtype: bf16 operands with f32 PSUM accumulation. Softmax skips
the max-subtraction (scores are < ~15 at this problem's scale), the
denominator comes free from an appended ones-column in the PV matmul,
and normalization is applied to the [tok, 128] attention output instead
of the [tok, 2048] probabilities.

RoPE layout trick: wq/wk columns are host-permuted so each head's even
dims come first and odd dims second. The rotation's pair swap then
becomes two 64-partition block copies (SBUF->SBUF DMA) instead of a
cross-partition interleave.
"""

import math
from types import SimpleNamespace

import numpy as np
import ml_dtypes

P = 128
BF16 = ml_dtypes.bfloat16


_CACHE = {}
_TRACE = False


def make_cfg(B=2, S=2048, DIM=4096, H=32, KVH=8, HD=128, THETA=500000.0,
             NCORES=8):
    c = SimpleNamespace(B=B, S=S, DIM=DIM, H=H, KVH=KVH, HD=HD, THETA=THETA,
                        NCORES=NCORES)
    c.T = B * S
    c.HPC = H // NCORES          # query heads per core
    c.QF = c.HPC * HD            # query features per core
    c.SCALE = 1.0 / math.sqrt(HD)
    c.TCH = 512                  # token chunk
    c.NKT = DIM // P             # contraction tiles
    c.NTT = c.T // P             # token tiles
    c.NCH = c.T // c.TCH         # token chunks
    c.SQT = S // P               # q/k tiles per sequence
    c.VW = HD + 1                # v + ones column
    c.AF = H * HD                # total attention features (wo rows)
    c.OF = DIM // NCORES         # output columns per core
    assert S % c.TCH == 0 and c.T % c.TCH == 0 and DIM % P == 0
    # each core's HPC query heads share the core's single kv head
    assert KVH == NCORES and c.HPC == H // KVH
    return c


def _build_graph(c, phases=4):
    """Build + compile the SPMD Bass graph (same program on every core)."""
    import concourse.mybir as mybir
    import concourse.tile as tile
    from concourse import bacc

    fp32 = mybir.dt.float32
    bf16 = mybir.dt.bfloat16

    nc = bacc.Bacc(
        "TRN2",
        target_bir_lowering=False,
        debug=False,
        enable_asserts=True,
        num_devices=c.NCORES,
    )

    # ---- kernel I/O ----
    xT = nc.dram_tensor("xT", [c.DIM, c.T], bf16, kind="ExternalInput").ap()
    wq = nc.dram_tensor("wq", [c.DIM, c.QF], bf16, kind="ExternalInput").ap()
    wk = nc.dram_tensor("wk", [c.DIM, c.HD], bf16, kind="ExternalInput").ap()
    wv = nc.dram_tensor("wv", [c.DIM, c.HD], bf16, kind="ExternalInput").ap()
    wo = nc.dram_tensor("wo", [c.AF, c.OF], bf16, kind="ExternalInput").ap()
    cosi = nc.dram_tensor("cosi", [P, c.T], fp32, kind="ExternalInput").ap()
    sini = nc.dram_tensor("sini", [P, c.T], fp32, kind="ExternalInput").ap()
    tril = nc.dram_tensor("tril", [P, P], bf16, kind="ExternalInput").ap()
    ident = nc.dram_tensor("ident", [P, P], fp32, kind="ExternalInput").ap()
    out = nc.dram_tensor("out", [c.T, c.OF], fp32, kind="ExternalOutput").ap()

    Exp = mybir.ActivationFunctionType.Exp
    Copy = mybir.ActivationFunctionType.Copy
    TPP = c.TCH // P          # token sub-tiles per chunk
    NQT = c.HPC + 1           # rope targets per chunk: HPC q tiles + 1 k
    SPB = c.S // P            # 128-token tiles per batch
    CPB = c.NCH // c.B        # token chunks per batch
    KG = 8                    # contraction tiles fetched per DMA

    with tile.TileContext(nc) as tc:
        # ------- static SBUF tensors (split per batch so attention of
        # batch 0 can start while batch 1 is still projecting) -------
        qT_b, kT_b, v_b, free_stat = [], [], [], []
        for b in range(c.B):
            t_, f_ = tc.tile([P, c.HPC, c.S], bf16, name=f"qT_sb{b}")
            qT_b.append(t_); free_stat.append(f_)
            t_, f_ = tc.tile([P, c.S], bf16, name=f"kT_sb{b}")
            kT_b.append(t_); free_stat.append(f_)
            t_, f_ = tc.tile([P, SPB, c.VW], bf16, name=f"v_sb{b}")
            v_b.append(t_); free_stat.append(f_)
        tril_sb, free_tril = tc.tile([P, P], bf16, name="tril_sb")
        id_sb, free_id = tc.tile([P, P], fp32, name="id_sb")
        free_stat += [free_tril, free_id]

        nc.sync.dma_start(tril_sb[:], tril[:])
        nc.sync.dma_start(id_sb[:], ident[:])
        for b in range(c.B):
            nc.vector.memset(v_b[b][:, :, c.HD:c.VW], 1.0)  # denominator ones

        with tc.tile_pool(name="dram", bufs=1, space="DRAM") as dramp:
            attnT_b = [
                dramp.tile([c.QF, c.S], bf16, name=f"attnT{b}")
                for b in range(c.B)
            ]
            gathered_b = [
                dramp.tile([c.NCORES * c.QF, c.S], bf16, addr_space="Shared",
                           name=f"gathered{b}")
                for b in range(c.B)
            ]

            # ============ Phase 1: projections + RoPE ============
            with tc.tile_pool(name="wpool", bufs=1) as wpool, \
                 tc.tile_pool(name="xpool", bufs=3) as xpool, \
                 tc.tile_pool(name="tabs", bufs=2) as tabs, \
                 tc.tile_pool(name="rope", bufs=2) as ropep, \
                 tc.tile_pool(name="pj_ps", bufs=1, space="PSUM") as pjps:

                # per-kt weight tiles, emitted just-in-time inside the
                # first chunk's kg loop (below) so the queues deliver each
                # kt's weights right before its first matmul instead of
                # front-loading 12 MB ahead of the x stream
                wq_t, wk_t, wv_t = [None] * c.NKT, [None] * c.NKT, [None] * c.NKT

                def load_weights(kt):
                    wqt = wpool.tile([P, c.QF], bf16, tag="wq", bufs=c.NKT,
                                     name=f"wq_t{kt}")
                    nc.sync.dma_start(wqt[:], wq[kt * P:(kt + 1) * P, :])
                    wq_t[kt] = wqt
                    wkt = wpool.tile([P, c.HD], bf16, tag="wk", bufs=c.NKT,
                                     name=f"wk_t{kt}")
                    nc.gpsimd.dma_start(wkt[:], wk[kt * P:(kt + 1) * P, :])
                    wk_t[kt] = wkt
                    wvt = wpool.tile([P, c.HD], bf16, tag="wv", bufs=c.NKT,
                                     name=f"wv_t{kt}")
                    nc.gpsimd.dma_start(wvt[:], wv[kt * P:(kt + 1) * P, :])
                    wv_t[kt] = wvt

                for ch in range(c.NCH):
                    t0 = ch * c.TCH
                    bch = ch // CPB           # batch of this chunk
                    lt0 = t0 - bch * c.S      # batch-local token offset
                    q_ps = [
                        pjps.tile([P, c.TCH], fp32, tag=f"q{ft}", bufs=1,
                                  name=f"q_ps{ft}")
                        for ft in range(c.HPC)
                    ]
                    k_ps = pjps.tile([P, c.TCH], fp32, tag="k", bufs=1)
                    v_ps = pjps.tile([P, c.TCH], fp32, tag="v", bufs=1)

                    for kg in range(c.NKT // KG):
                        if ch == 0:
                            for kt in range(kg * KG, (kg + 1) * KG):
                                load_weights(kt)
                        # one DMA brings KG=4 contraction tiles (512 KB)
                        xt4 = xpool.tile([P, KG, c.TCH], bf16, tag="xt")
                        nc.sync.dma_start(
                            xt4[:],
                            xT[kg * KG * P:(kg + 1) * KG * P,
                               t0:t0 + c.TCH].rearrange(
                                   "(o p) t -> p o t", p=P),
                        )
                        for ki in range(KG):
                            kt = kg * KG + ki
                            xt = xt4[:, ki, :]
                            st = kt == 0
                            sp = kt == c.NKT - 1
                            for ft in range(c.HPC):
                                nc.tensor.matmul(
                                    q_ps[ft][:],
                                    lhsT=wq_t[kt][:, ft * P:(ft + 1) * P],
                                    rhs=xt,
                                    start=st, stop=sp,
                                )
                            nc.tensor.matmul(
                                k_ps[:], lhsT=wk_t[kt][:], rhs=xt,
                                start=st, stop=sp,
                            )
                            # vT (feature-major); PE-transposed below
                            nc.tensor.matmul(
                                v_ps[:], lhsT=wv_t[kt][:], rhs=xt,
                                start=st, stop=sp,
                            )

                    # vT -> SBUF f32, PE-transpose to token-major, cast bf16
                    vt_sb = ropep.tile([P, c.TCH], fp32, tag="vt", name="vt_sb")
                    nc.scalar.activation(vt_sb[:], v_ps[:], Copy)
                    for sub in range(TPP):
                        gt = lt0 // P + sub
                        vtp = pjps.tile([P, P], fp32, tag="vtp", bufs=2,
                                        name="vtp")
                        nc.tensor.transpose(
                            vtp[:], vt_sb[:, sub * P:(sub + 1) * P], id_sb[:]
                        )
                        nc.vector.tensor_copy(v_b[bch][:, gt, 0:c.HD], vtp[:])

                    # ---- RoPE on all q tiles + k at once (mega-tile) ----
                    ct = tabs.tile([P, c.TCH], fp32, tag="cos")
                    st_t = tabs.tile([P, c.TCH], fp32, tag="sin")
                    nc.sync.dma_start(ct[:], cosi[:, t0:t0 + c.TCH])
                    nc.sync.dma_start(st_t[:], sini[:, t0:t0 + c.TCH])

                    qbig = ropep.tile([P, NQT, c.TCH], fp32, tag="qbig",
                                      name="qbig")
                    # psum -> sbuf copies split across ACT and DVE so the
                    # accumulators free quickly for the next chunk
                    for ft in range(c.HPC):
                        eng = nc.scalar if ft % 2 == 0 else nc.vector
                        if ft % 2 == 0:
                            nc.scalar.activation(qbig[:, ft, :], q_ps[ft][:],
                                                 Copy)
                        else:
                            nc.vector.tensor_copy(qbig[:, ft, :], q_ps[ft][:])
                    nc.scalar.activation(qbig[:, c.HPC, :], k_ps[:], Copy)

                    qsw = ropep.tile([P, NQT, c.TCH], fp32, tag="qsw",
                                     name="qsw")
                    # pair swap == half-partition block swap (even|odd split)
                    nc.sync.dma_start(qsw[0:64, :, :], qbig[64:128, :, :])
                    nc.sync.dma_start(qsw[64:128, :, :], qbig[0:64, :, :])

                    ctb = ct[:, None, :].to_broadcast((P, NQT, c.TCH))
                    stb = st_t[:, None, :].to_broadcast((P, NQT, c.TCH))
                    nc.vector.tensor_mul(qbig[:], qbig[:], ctb)
                    nc.vector.tensor_mul(qsw[:], qsw[:], stb)
                    rr = ropep.tile([P, NQT, c.TCH], bf16, tag="rr", name="rr")
                    nc.vector.tensor_add(rr[:], qbig[:], qsw[:])
                    for ft in range(c.HPC):
                        nc.vector.tensor_copy(
                            qT_b[bch][:, ft, lt0:lt0 + c.TCH], rr[:, ft, :])
                    nc.vector.tensor_copy(
                        kT_b[bch][:, lt0:lt0 + c.TCH], rr[:, c.HPC, :])

            # ============ Phase 2: attention ============
            if phases >= 2:
                with tc.tile_pool(name="spool", bufs=3, space="PSUM") as spool, \
                     tc.tile_pool(name="opool", bufs=2, space="PSUM") as opool, \
                     tc.tile_pool(name="tpool", bufs=2, space="PSUM") as tpool, \
                     tc.tile_pool(name="ppool", bufs=32) as ppool, \
                     tc.tile_pool(name="apool", bufs=4) as apool:

                    for b in range(c.B):
                        for h in range(c.HPC):
                            qh = qT_b[b][:, h, :]
                            pts = {}
                            atst = {}
                            for ki in range(c.SQT):
                                q0 = ki * P
                                nch = (c.S - q0 + c.TCH - 1) // c.TCH
                                for m in range(nch):
                                    c0 = q0 + m * c.TCH
                                    w = min(c.TCH, c.S - c0)
                                    s_ps = spool.tile([P, c.TCH], fp32,
                                                      tag="s", name="s_ps")
                                    nc.tensor.matmul(
                                        s_ps[:, :w],
                                        lhsT=kT_b[b][:, q0:q0 + P],
                                        rhs=qh[:, c0:c0 + w],
                                        start=True, stop=True,
                                    )
                                    pt = ppool.tile([P, c.TCH], bf16,
                                                    tag="pt", name="pt")
                                    nc.scalar.activation(
                                        pt[:, :w], s_ps[:, :w], Exp,
                                        scale=c.SCALE
                                    )
                                    if m == 0:
                                        nc.vector.tensor_mul(
                                            pt[:, 0:P], pt[:, 0:P], tril_sb[:]
                                        )
                                    pts[(ki, m)] = pt
                                # ---- PV for qi == ki ----
                                qi = ki
                                o_ps = opool.tile([P, c.VW], fp32, tag="o",
                                                  name="o_ps")
                                for kj in range(qi + 1):
                                    m = (qi - kj) // TPP
                                    off = ((qi - kj) % TPP) * P
                                    nc.tensor.matmul(
                                        o_ps[:],
                                        lhsT=pts[(kj, m)][:, off:off + P],
                                        rhs=v_b[b][:, kj, :],
                                        start=(kj == 0), stop=(kj == qi),
                                    )
                                rec = apool.tile([P, 1], fp32, tag="rec",
                                                 name="rec")
                                nc.vector.reciprocal(rec[:], o_ps[:, c.HD:c.VW])
                                ao = apool.tile([P, P], fp32, tag="ao",
                                                name="ao")
                                nc.vector.tensor_scalar_mul(
                                    ao[:], o_ps[:, 0:c.HD], rec[:]
                                )
                                tp = tpool.tile([P, P], fp32, tag="tp",
                                                name="tp")
                                nc.tensor.transpose(tp[:], ao[:], id_sb[:])
                                # stage 4 consecutive qi into one tile so the
                                # attnT write is one 128 KB DMA instead of 4
                                g4 = qi // TPP
                                if qi % TPP == 0:
                                    atst[g4] = apool.tile(
                                        [P, c.TCH], bf16, tag="at4",
                                        bufs=2, name="at4")
                                nc.vector.tensor_copy(
                                    atst[g4][:, (qi % TPP) * P:
                                             (qi % TPP + 1) * P], tp[:])
                                if qi % TPP == TPP - 1:
                                    nc.sync.dma_start(
                                        attnT_b[b][h * P:(h + 1) * P,
                                                   g4 * c.TCH:
                                                   (g4 + 1) * c.TCH],
                                        atst[g4][:],
                                    )

            # ============ Phase 3: AllGather attention outputs ============
            if phases >= 3:
                for b in range(c.B):
                    nc.gpsimd.collective_compute(
                        "AllGather",
                        mybir.AluOpType.bypass,
                        replica_groups=[list(range(c.NCORES))],
                        ins=[attnT_b[b][:].opt()],
                        outs=[gathered_b[b][:].opt()],
                    )

            # ============ Phase 4: output projection (column shard) ============
            if phases >= 4:
                with tc.tile_pool(name="wop", bufs=1) as wop, \
                     tc.tile_pool(name="atp", bufs=3) as atp, \
                     tc.tile_pool(name="obp", bufs=4) as obp, \
                     tc.tile_pool(name="wo_ps", bufs=8, space="PSUM") as wops:

                    wo_t = []
                    for f in range(c.AF // P):
                        wot = wop.tile([P, c.OF], bf16, tag="wo",
                                       bufs=c.AF // P, name=f"wo_t{f}")
                        nc.gpsimd.dma_start(wot[:], wo[f * P:(f + 1) * P, :])
                        wo_t.append(wot)
                    NFT = c.AF // P  # feature tiles of gathered attnout

                    for ch in range(c.NCH):
                        t0 = ch * c.TCH
                        bch = ch // CPB
                        lt0 = t0 - bch * c.S
                        o_ps = [
                            wops.tile([P, c.OF], fp32, tag=f"wo{sub}", bufs=2,
                                      name=f"wo_ps{sub}")
                            for sub in range(TPP)
                        ]
                        for fg in range(NFT // KG):
                            at4 = atp.tile([P, KG, c.TCH], bf16, tag="at")
                            nc.sync.dma_start(
                                at4[:],
                                gathered_b[bch][fg * KG * P:(fg + 1) * KG * P,
                                                lt0:lt0 + c.TCH].rearrange(
                                                    "(o p) t -> p o t", p=P),
                            )
                            for fi in range(KG):
                                f = fg * KG + fi
                                for sub in range(TPP):
                                    nc.tensor.matmul(
                                        o_ps[sub][:],
                                        lhsT=at4[:, fi,
                                                 sub * P:(sub + 1) * P],
                                        rhs=wo_t[f][:],
                                        start=(f == 0), stop=(f == NFT - 1),
                                    )
                        for sub in range(TPP):
                            ob = obp.tile([P, c.OF], fp32, tag="ob", name="ob")
                            nc.scalar.activation(ob[:], o_ps[sub][:], Copy)
                            nc.sync.dma_start(
                                out[(ch * TPP + sub) * P:
                                    (ch * TPP + sub + 1) * P, :],
                                ob[:],
                            )

        # release static single-tile pools in LIFO order
        for f_ in reversed(free_stat):
            f_()

    nc.compile()
    return nc


def _host_inputs(c, x, wq, wk, wv, wo):
    """Shard + lay out the inputs for the cores."""
    xT = np.ascontiguousarray(x.reshape(c.T, c.DIM).T).astype(BF16)

    # even/odd split permutation within each head (q and k only)
    perm_head = np.concatenate([np.arange(0, c.HD, 2), np.arange(1, c.HD, 2)])

    def permute_heads(w):  # w: [DIM, n*HD]
        nh = w.shape[1] // c.HD
        w = w.reshape(c.DIM, nh, c.HD)[:, :, perm_head]
        return np.ascontiguousarray(w.reshape(c.DIM, nh * c.HD))

    wq_p = permute_heads(wq).astype(BF16)
    wk_p = permute_heads(wk).astype(BF16)
    wv_b = wv.astype(BF16)
    wo_b = wo.astype(BF16)

    # rope tables, even/odd-split feature-major layout: [128, T]
    hh = c.HD // 2
    inv = 1.0 / (c.THETA ** (np.arange(0, c.HD, 2, dtype=np.float64) / c.HD))
    pos = (np.arange(c.T) % c.S).astype(np.float64)
    ang = inv[:, None] * pos[None, :]              # [64, T]
    cosv = np.cos(ang).astype(np.float32)
    sinv = np.sin(ang).astype(np.float32)
    cosi = np.concatenate([cosv, cosv], 0)
    sini = np.concatenate([-sinv, sinv], 0)
    assert hh * 2 == P

    trilm = np.ascontiguousarray(
        np.tril(np.ones((P, P), np.float32)).T
    ).astype(BF16)                                  # [k, q]: 1 iff k<=q
    identm = np.eye(P, dtype=np.float32)

    KHC = c.KVH // c.NCORES  # kv heads per core (=1)
    in_maps = []
    for cc in range(c.NCORES):
        in_maps.append({
            "xT": xT,
            "wq": np.ascontiguousarray(wq_p[:, cc * c.QF:(cc + 1) * c.QF]),
            "wk": np.ascontiguousarray(
                wk_p[:, cc * KHC * c.HD:(cc * KHC + 1) * c.HD]),
            "wv": np.ascontiguousarray(
                wv_b[:, cc * KHC * c.HD:(cc * KHC + 1) * c.HD]),
            "wo": np.ascontiguousarray(wo_b[:, cc * c.OF:(cc + 1) * c.OF]),
            "cosi": cosi,
            "sini": sini,
            "tril": trilm,
            "ident": identm,
        })
    return in_maps


def assemble(c, outs):
    full = np.concatenate(outs, axis=1).astype(np.float32)
    return full.reshape(c.B, c.S, c.DIM)


def kernel(x, wq, wk, wv, wo):
    from concourse import bass_utils

    if "nc" not in _CACHE:
        _CACHE["cfg"] = make_cfg()
        _CACHE["nc"] = _build_graph(_CACHE["cfg"])
    nc = _CACHE["nc"]
    c = _CACHE["cfg"]

    in_maps = _host_inputs(
        c, np.asarray(x), np.asarray(wq), np.asarray(wk),
        np.asarray(wv), np.asarray(wo),
    )
    res = bass_utils.run_bass_kernel_spmd(
        nc, in_maps, core_ids=list(range(c.NCORES)), trace=_TRACE
    )
    _CACHE["last_results"] = res
    outs = [res.results[i]["out"] for i in range(c.NCORES)]
    return assemble(c, outs)
